# revision 40
# baseline (speedup 1.0000x reference)
"""ChannelAttention Trainium2 kernel (Bass/Tile), data-parallel over batch.

Problem shapes (hardcoded):
  x      [8, 4096, 768] fp32
  w_qkv  [2304, 768]    fp32
  w_proj [768, 768]     fp32
  b_proj [768]          fp32
  out    [8, 4096, 768] fp32

Reference (per batch b, 8 groups of 96 channels):
  qkv = x @ w_qkv.T ; q *= N**-0.5
  attn_g = softmax(q_g.T @ k_g, axis=-1)     # [96, 96], contracts over N
  out_g  = attn_g @ v_g.T                    # [96, N]
  y = out @ w_proj.T + b_proj

Sharding: batch b -> core b (8 cores SPMD, no collectives).

Algebraic restructure: channel attention collapses around two small
matrices --
  G = X^T X                      [768, 768]   (Gram, symmetric)
  attn_g = softmax(Wq_s G Wk^T)  (per group, [96, 96])
  M = Wv^T BD(attn)^T WprojT     [768, 768]
  y = x @ M + b_proj
so the per-token work is ONE 768-contraction pass for G (using x in
natural layout) and ONE for y (using x^T), plus O(768^3)-ish small
matmuls once per core. All matmul operands fp16 (full PE rate), fp32
accumulation in PSUM; softmax in fp32.

v5 changes vs v4:
  - x^T comes from the host (layout prep) instead of on-chip PE
    transposes: saves ~25k PE cycles + 192 PSUM->SBUF copies.
  - Gram accumulates directly in PSUM across all 32 token tiles
    (8 persistent [128,512] fp32 banks = 8 upper-block column windows):
    no per-supertile DVE adds, G32 SBUF dropped.
  - Lower G blocks mirrored from the fp16 cast (PE transpose at fp16
    rate, not fp32).
  - y stored as fp16 (host upcasts): halves output DMA.
v6 changes vs v5:
  - Per-queue DMA is packet-rate limited (~35-45 packets/us), so
    1536B-row x tiles cap a queue near 60 GB/s.  x is repacked on the
    host as supertiles [8, 128, 3072]: partition p of supertile s holds
    4 consecutive tokens (6KB contiguous HBM per packet).  The Gram is
    token-permutation invariant (both matmul operands come from the
    same tile), so phase 1 just consumes the 4 column sub-tiles of each
    supertile.  Each supertile DMA is split into two 64-partition
    halves on separate queues.
Host pre-work: fp16 casts, fold N**-0.5 into Wq, transposes of x, the
q/k weight halves and of w_proj (layout prep only).
"""

import numpy as np

B, N, C = 8, 4096, 768
G = 8
GC = C // G          # 96
NCORES = 8
NST = 8              # x supertiles (512 tokens each, 4 per partition)
NSUB = 4             # column sub-tiles per supertile
NT = N // 128        # 32 token tiles
CC = C // 128        # 6 chunks of the channel dim
QSCALE = float(N) ** -0.5  # 1/64

# Gram upper-block column windows per row-chunk a: (off, w, bank).
# PSUM accumulation "zero regions" are whole 2KB banks (a start=True
# zeroes the full bank), so each concurrently-accumulating window owns
# one [128, 512] fp32 bank.  The upper triangle is 2688 fp32 columns ->
# exactly 8 windows of <=512 in the 8 banks.
GRAM_WINDOWS = [
    [(0, 512, 0), (512, 256, 1)],
    [(128, 384, 2), (512, 256, 3)],
    [(256, 512, 4)],
    [(384, 384, 5)],
    [(512, 256, 6)],
    [(640, 128, 7)],
]

_CACHE = {}


def _build_nc():
    import concourse.bass as bass
    import concourse.mybir as mybir
    import concourse.tile as tile
    from concourse import bacc

    fp16 = mybir.dt.float16
    fp32 = mybir.dt.float32

    nc = bacc.Bacc(
        "TRN2", target_bir_lowering=False, debug=False, num_devices=NCORES
    )

    xh = nc.dram_tensor("xh", [NST, 128, NSUB * C], fp16, kind="ExternalInput").ap()
    xhT = nc.dram_tensor("xhT", [C, N], fp16, kind="ExternalInput").ap()
    # q/k halves of w_qkv, transposed to [c, 2*768], q pre-scaled
    wqkT = nc.dram_tensor("wqkT", [C, 2 * C], fp16, kind="ExternalInput").ap()
    # v rows of w_qkv in natural [d, a] layout
    wv = nc.dram_tensor("wv", [C, C], fp16, kind="ExternalInput").ap()
    wprojT = nc.dram_tensor("wprojT", [C, C], fp16, kind="ExternalInput").ap()
    bproj = nc.dram_tensor("bproj", [C], fp32, kind="ExternalInput").ap()
    id16d = nc.dram_tensor("id16", [128, 128], fp16, kind="ExternalInput").ap()
    y = nc.dram_tensor("y", [N, C], fp16, kind="ExternalOutput").ap()

    with tile.TileContext(nc) as tc:
        from contextlib import ExitStack

        with ExitStack() as ctx:
            persist = ctx.enter_context(tc.tile_pool(name="persist", bufs=1))
            xn_pool = ctx.enter_context(tc.tile_pool(name="xn", bufs=6))
            stream = ctx.enter_context(tc.tile_pool(name="stream", bufs=8))
            weights = persist
            ysb_pool = stream
            sm_pool = stream

            # ---- static weight tiles ----
            wqk_sb = [
                weights.tile([128, 2 * C], fp16, name=f"wqk_{a}")
                for a in range(CC)
            ]
            wv_sb = [
                weights.tile([128, C], fp16, name=f"wv_{dd}") for dd in range(CC)
            ]
            wpg_sb = [
                weights.tile([GC, C], fp16, name=f"wpg_{g}") for g in range(G)
            ]
            bias_sb = weights.tile([128, C], fp32, name="bias_sb")
            ident16 = weights.tile([128, 128], fp16, name="ident16")

            # ---- persistent intermediates ----
            G16 = [
                persist.tile([128, C], fp16, name=f"G16_{a}") for a in range(CC)
            ]
            xT6 = [
                persist.tile([128, N], fp16, name=f"xT_{a}") for a in range(CC)
            ]
            e16 = [
                persist.tile([GC, GC], fp16, name=f"e16_{g}") for g in range(G)
            ]
            P6 = [persist.tile([128, C], fp16, name=f"P_{dd}") for dd in range(CC)]
            M_sb = [
                persist.tile([128, C], fp16, name=f"M_{a}") for a in range(CC)
            ]
            M1_sb = [
                persist.tile([128, C], fp16, name=f"m1_{a}") for a in range(CC)
            ]

            # ---- phase 1: stream x, accumulate Gram upper blocks into
            # persistent PSUM across all 32 token tiles ----
            with tc.tile_pool(name="ps_gram", bufs=1, space="PSUM") as ps_gram:
                g_ps = [
                    ps_gram.tile([128, 512], fp32, name=f"gps_{i}")
                    for i in range(8)
                ]

                xn = []
                for s in range(NST):
                    xtile = xn_pool.tile(
                        [128, NSUB * C], fp16, tag="xn", name=f"xn_{s}"
                    )
                    # split each supertile into two 64-partition halves on
                    # separate queues (6KB packets either way).  The first
                    # supertile splits by COLUMNS instead: subtiles 0-1
                    # unlock after a single transfer (one 16-engine
                    # completion instead of four), so compute starts sooner.
                    if s == 0:
                        # 2x2 pieces: each column-half arrives as one
                        # 64-packet transfer per queue, so subtiles 0-1
                        # unlock ~1.6us after the queues start
                        for cl in range(2):
                            cs = slice(cl * 2 * C, (cl + 1) * 2 * C)
                            nc.scalar.dma_start(
                                out=xtile[0:64, cs], in_=xh[s][0:64, cs]
                            )
                            nc.sync.dma_start(
                                out=xtile[64:128, cs], in_=xh[s][64:128, cs]
                            )
                    else:
                        nc.scalar.dma_start(
                            out=xtile[0:64, :], in_=xh[s][0:64, :]
                        )
                        nc.sync.dma_start(
                            out=xtile[64:128, :], in_=xh[s][64:128, :]
                        )
                    xn.append(xtile)

                # weight loads (needed from phase 2 on) on the gpsimd
                # queue.  Delayed so their packets don't sit on the shared
                # DMA engines during the first supertile loads: the PE's
                # first wait needs every one of a transfer's 16
                # engine-completions, and a straggler engine with queued
                # weight packets costs ~3us at kernel start.
                with tc.tile_wait_until(0.025):
                    for a in range(CC):
                        nc.gpsimd.dma_start(
                            out=wqk_sb[a], in_=wqkT[a * 128 : (a + 1) * 128, :]
                        )
                    for g in range(G):
                        nc.gpsimd.dma_start(
                            out=wpg_sb[g], in_=wprojT[g * GC : (g + 1) * GC, :]
                        )
                    for dd in range(CC):
                        nc.gpsimd.dma_start(
                            out=wv_sb[dd], in_=wv[dd * 128 : (dd + 1) * 128, :]
                        )
                bias_bcast = bass.AP(
                    tensor=bproj.tensor,
                    offset=bproj.offset,
                    ap=[[0, 128]] + [list(p) for p in bproj.ap],
                )
                # x^T loads: manually push these 1MB transfers after the
                # phase-1 x stream on the same queues (the scheduler would
                # otherwise hoist them and starve the PE of supertiles).
                # ident16 (slow 256B-packet DMA, needed at ~50us for the
                # mirrors) goes on sync ahead of the x^T tiles.
                with tc.tile_wait_until(0.030):
                    nc.sync.dma_start(out=ident16, in_=id16d)
                    for a in range(CC):
                        eng = nc.sync if a % 2 == 0 else nc.scalar
                        eng.dma_start(
                            out=xT6[a], in_=xhT[a * 128 : (a + 1) * 128, :]
                        )
                    nc.gpsimd.dma_start(out=bias_sb, in_=bias_bcast)

                def xsl(t, off, w):
                    s, j = t // NSUB, t % NSUB
                    base = j * C + off
                    return xn[s][:, base : base + w]

                for t in range(NT):
                    first, last = (t == 0), (t == NT - 1)
                    for a in range(CC):
                        lhs = xsl(t, a * 128, 128)
                        for (off, w, pi) in GRAM_WINDOWS[a]:
                            nc.tensor.matmul(
                                g_ps[pi][:, 0:w],
                                lhs,
                                xsl(t, off, w),
                                start=first,
                                stop=last,
                            )

                # cast upper blocks PSUM -> fp16 (emitted right after the
                # final tile's matmuls; each window drains as its chain
                # stops)
                k = 0
                for a in range(CC):
                    for (off, w, pi) in GRAM_WINDOWS[a]:
                        if k % 2:
                            nc.vector.tensor_copy(
                                G16[a][:, off : off + w], g_ps[pi][:, 0:w]
                            )
                        else:
                            nc.scalar.copy(
                                out=G16[a][:, off : off + w],
                                in_=g_ps[pi][:, 0:w],
                            )
                        k += 1

            with tc.tile_pool(name="ps_big", bufs=5, space="PSUM") as ps_big:
                # ---- phase 2a: mirror lower G16 blocks by PE transpose of
                # the fp16 upper blocks ----
                for b_ in range(1, CC):
                    for a in range(b_):
                        m_ps = ps_big.tile(
                            [128, 128], fp16, tag="big", name=f"mir_{a}_{b_}"
                        )
                        nc.tensor.transpose(
                            m_ps, G16[a][:, b_ * 128 : (b_ + 1) * 128], ident16
                        )
                        if (a + b_) % 2:
                            nc.vector.tensor_copy(
                                G16[b_][:, a * 128 : (a + 1) * 128], m_ps
                            )
                        else:
                            nc.scalar.copy(
                                out=G16[b_][:, a * 128 : (a + 1) * 128],
                                in_=m_ps,
                            )

                # ---- phase 2b: M1 = G Wk^T (all groups batched), then per
                # group A_g = Wq_s_g^T M1_g, softmax ----
                for a in range(CC):
                    for half in range(2):
                        hsl = slice(half * 384, (half + 1) * 384)
                        m1_ps = ps_big.tile(
                            [128, 384], fp32, tag="big", name=f"m1ps_{a}_{half}"
                        )
                        for b_ in range(CC):
                            nc.tensor.matmul(
                                m1_ps,
                                G16[b_][:, a * 128 : (a + 1) * 128],
                                wqk_sb[b_][
                                    :, 768 + half * 384 : 768 + (half + 1) * 384
                                ],
                                start=(b_ == 0),
                                stop=(b_ == CC - 1),
                            )
                        if (a + half) % 2 == 0:
                            nc.scalar.copy(out=M1_sb[a][:, hsl], in_=m1_ps)
                        else:
                            nc.vector.tensor_copy(M1_sb[a][:, hsl], m1_ps)

                for g in range(G):
                    a_ps = ps_big.tile([GC, GC], fp32, tag="big", name=f"aps_{g}")
                    for a in range(CC):
                        nc.tensor.matmul(
                            a_ps,
                            wqk_sb[a][:, g * GC : (g + 1) * GC],
                            M1_sb[a][:, g * GC : (g + 1) * GC],
                            start=(a == 0),
                            stop=(a == CC - 1),
                        )

                    nm = sm_pool.tile([GC, 1], fp32, tag="nm", name=f"nm_{g}")
                    nc.vector.tensor_reduce(
                        out=nm,
                        in_=a_ps,
                        axis=mybir.AxisListType.X,
                        op=mybir.AluOpType.max,
                        negate=True,
                    )
                    e_t = sm_pool.tile([GC, GC], fp32, tag="e", name=f"e_{g}")
                    ssum = sm_pool.tile([GC, 1], fp32, tag="ssum", name=f"ssum_{g}")
                    nc.scalar.activation(
                        e_t,
                        a_ps,
                        mybir.ActivationFunctionType.Exp,
                        bias=nm,
                        scale=1.0,
                        accum_out=ssum,
                    )
                    rs = sm_pool.tile([GC, 1], fp32, tag="rs", name=f"rs_{g}")
                    nc.vector.reciprocal(rs, ssum)
                    nc.vector.tensor_scalar_mul(e16[g], e_t, rs)

                # ---- phase 2c: P = BD(attn)^T WprojT in 128-aligned d-chunks
                # (piece matmuls land at their global-d psum partitions via
                # tile_position col offsets) ; M = Wv^T P with K=128 ----
                def d_pieces(dd):
                    raw = []
                    for g in range(G):
                        lo, hi = g * GC, (g + 1) * GC
                        r0 = max(0, 128 * dd - lo)
                        r1 = min(GC, 128 * (dd + 1) - lo)
                        if r0 < r1:
                            raw.append((g, r0, r1, lo + r0 - 128 * dd))
                    # split pieces that violate PE col-group placement rules
                    # (M<=32 at {0,32,64,96}; M<=64 at {0,64}; M>64 only at 0)
                    out = []
                    for (g, r0, r1, p0) in raw:
                        while r0 < r1:
                            m = r1 - r0
                            if p0 == 0 or (m <= 32) or (m <= 64 and p0 == 64):
                                out.append((g, r0, r1, p0))
                                break
                            step = 32 if p0 % 64 else 64
                            step = min(step, m)
                            out.append((g, r0, r0 + step, p0))
                            r0 += step
                            p0 += step
                    return out

                for dd in range(CC):
                    for half in range(2):
                        hsl = slice(half * 384, (half + 1) * 384)
                        p_ps = ps_big.tile(
                            [128, 384], fp32, tag="big", name=f"pps_{dd}_{half}"
                        )
                        for (g, r0, r1, p0) in d_pieces(dd):
                            nc.tensor.matmul(
                                p_ps[p0 : p0 + (r1 - r0), :],
                                e16[g][:, r0:r1],
                                wpg_sb[g][:, hsl],
                                start=True,
                                stop=True,
                                tile_position=(0, p0) if p0 else None,
                            )
                        if dd % 2 == 0:
                            nc.scalar.copy(out=P6[dd][:, hsl], in_=p_ps)
                        else:
                            nc.vector.tensor_copy(P6[dd][:, hsl], p_ps)

                for half in range(2):
                    for ab in range(CC):
                        hsl = slice(half * 384, (half + 1) * 384)
                        m_ps = ps_big.tile(
                            [128, 384], fp32, tag="big", name=f"mps_{ab}_{half}"
                        )
                        for dd in range(CC):
                            nc.tensor.matmul(
                                m_ps,
                                wv_sb[dd][:, ab * 128 : (ab + 1) * 128],
                                P6[dd][:, hsl],
                                start=(dd == 0),
                                stop=(dd == CC - 1),
                            )
                        if ab % 2 == 0:
                            nc.scalar.copy(out=M_sb[ab][:, hsl], in_=m_ps)
                        else:
                            nc.vector.tensor_copy(M_sb[ab][:, hsl], m_ps)

                # ---- phase 3: y = x @ M + b ----
                for t in range(NT):
                    r0 = t * 128
                    y_sb = ysb_pool.tile(
                        [128, C], fp16, tag="ysb", name=f"ysb_{t}"
                    )
                    for half in range(2):
                        hsl = slice(half * 384, (half + 1) * 384)
                        y_ps = ps_big.tile(
                            [128, 384], fp32, tag="big", name=f"yps_{t}_{half}"
                        )
                        for a in range(CC):
                            nc.tensor.matmul(
                                y_ps,
                                xT6[a][:, r0 : r0 + 128],
                                M_sb[a][:, hsl],
                                start=(a == 0),
                                stop=(a == CC - 1),
                            )
                        nc.vector.tensor_add(y_sb[:, hsl], y_ps, bias_sb[:, hsl])
                    if t >= NT - 4:
                        # drain the final tiles on both queues
                        nc.scalar.dma_start(
                            out=y[r0 : r0 + 64, :], in_=y_sb[0:64, :]
                        )
                        nc.sync.dma_start(
                            out=y[r0 + 64 : r0 + 128, :], in_=y_sb[64:128, :]
                        )
                    else:
                        # rotate across 3 queues so the output stream never
                        # builds a backlog (each tile is 128 packets; one
                        # queue moves ~40 packets/us)
                        out_eng = (nc.scalar, nc.sync, nc.gpsimd)[t % 3]
                        out_eng.dma_start(out=y[r0 : r0 + 128, :], in_=y_sb)

    nc.compile()
    return nc


def _get_nc():
    if "nc" not in _CACHE:
        _CACHE["nc"] = _build_nc()
    return _CACHE["nc"]


def _host_prep(x, w_qkv, w_proj, b_proj):
    x = np.asarray(x, dtype=np.float32)
    w_qkv = np.asarray(w_qkv, dtype=np.float32)
    w_proj = np.asarray(w_proj, dtype=np.float32)
    b_proj = np.asarray(b_proj, dtype=np.float32)

    wqk = w_qkv[: 2 * C, :].copy()
    wqk[:C, :] *= np.float32(QSCALE)
    wqkT_h = np.ascontiguousarray(wqk.T).astype(np.float16)       # [768, 1536]
    wv_h = np.ascontiguousarray(w_qkv[2 * C :, :]).astype(np.float16)
    wprojT_h = np.ascontiguousarray(w_proj.T).astype(np.float16)  # [768, 768]

    id16 = np.eye(128, dtype=np.float16)
    in_maps = []
    for b_ in range(NCORES):
        xb16 = x[b_].astype(np.float16)
        in_maps.append(
            {
                # supertile layout: [s, p, j*C+c] = x[s*512 + 4p + j, c]
                "xh": np.ascontiguousarray(xb16.reshape(NST, 128, NSUB * C)),
                "xhT": np.ascontiguousarray(xb16.T),
                "wqkT": wqkT_h,
                "wv": wv_h,
                "wprojT": wprojT_h,
                "bproj": b_proj,
                "id16": id16,
            }
        )
    return in_maps


def _run(in_maps, trace=False):
    from concourse.bass_utils import run_bass_kernel_spmd

    nc = _get_nc()
    res = run_bass_kernel_spmd(nc, in_maps, list(range(NCORES)), trace=trace)
    out = np.stack([res.results[i]["y"] for i in range(NCORES)], axis=0)
    return out.astype(np.float32, copy=False), res


def kernel(x, w_qkv, w_proj, b_proj):
    in_maps = _host_prep(x, w_qkv, w_proj, b_proj)
    out, _ = _run(in_maps, trace=False)
    return out


def run_profiled(x, w_qkv, w_proj, b_proj):
    """Returns (out, BassKernelResults) with NTFF profiling enabled."""
    in_maps = _host_prep(x, w_qkv, w_proj, b_proj)
    return _run(in_maps, trace=True)
